# revision 1
# baseline (speedup 1.0000x reference)
"""Trainium2 Bass kernel for nn_AttentionGCNLayer (B=2, N=4096, D=256, H=2, ITERS=2).

Sharding: 8 cores = (b in 2) x (h in 2) x (row-half in 2). Each core handles one
(batch, head) pair and one half (2048) of the attention rows, with a pairwise
AllGather of the updated node features between the two GCN iterations.

Layout choices (per core):
  - x is kept transposed (x^T, [D=2x128 partitions, N free]) in *local* row
    order: columns [0:2048) are this core's rows, [2048:4096) the partner's.
    The aggregation sum over neighbors is permutation invariant, so local
    ordering is consistent as long as k/h/E all use the same order (they do).
  - scores are computed transposed (E^T = exp(q k^T / sqrt(dk))^T with
    [neighbor n on partitions, attention rows on free]) so that the
    neighbor-aggregation matmul consumes E^T directly, with no transposes.
  - softmax normalizer: rows of exp(scores) are summed with a ones-vector
    matmul on the PE; attn @ h / degs == (E @ h) * (1/R) with R = rowsum(E)
    (degs == 1 up to fp rounding, matching the reference within fp32 noise).
    scores are in [-1, 1], so exp needs no max-subtraction.
  - big matmuls run in bf16 (inputs) with fp32 PSUM accumulation.
"""

import sys

if "/opt/trn_rl_repo" not in sys.path:
    sys.path.insert(0, "/opt/trn_rl_repo")

import numpy as np

B, N, D, H, ITERS = 2, 4096, 256, 2, 2
DK = D // H                      # 128
RH = N // 2                      # 2048 rows per core
NCH = N // 128                   # 32 neighbor chunks
HCH = NCH // 2                   # 16 chunks per half
RT = 512                         # row tile (one PSUM bank of fp32)
NRT = RH // RT                   # 4 row tiles per core
SCALE = 1.0 / float(np.sqrt(np.float32(DK)))

_CACHE = {}


def _seq_engines(mybir):
    return {
        mybir.EngineType.PE,
        mybir.EngineType.Activation,
        mybir.EngineType.Pool,
        mybir.EngineType.DVE,
        mybir.EngineType.SP,
    }


def _split_excess_waits(nc, mybir, max_waits=1):
    """This container's walrus accepts at most one sync-wait per engine
    instruction; hoist extra waits onto preceding NoOps on the same engine."""
    seq = _seq_engines(mybir)
    n_new = 0
    for f in nc.m.functions:
        for blk in f.blocks:
            if not any(
                inst.sync_info is not None
                and inst.sync_info.on_wait
                and len(inst.sync_info.on_wait) > max_waits
                and inst.engine in seq
                for inst in blk.instructions
            ):
                continue
            out = []
            for inst in blk.instructions:
                si = inst.sync_info
                if (
                    si is not None
                    and si.on_wait
                    and len(si.on_wait) > max_waits
                    and inst.engine in seq
                ):
                    waits = list(si.on_wait)
                    keep, extra = waits[:max_waits], waits[max_waits:]
                    while extra:
                        chunk, extra = extra[:max_waits], extra[max_waits:]
                        out.append(
                            mybir.InstNoOp(
                                name=f"{inst.name}-ws{n_new}",
                                sync_info=mybir.SyncInfo(on_wait=chunk, on_update=[]),
                                bass_nofuse=True,
                                engine=inst.engine,
                            )
                        )
                        n_new += 1
                    inst.sync_info = mybir.SyncInfo(
                        on_wait=keep, on_update=list(si.on_update)
                    )
                out.append(inst)
            blk.instructions = out
    return n_new


def _build():
    import concourse.bass as bass
    import concourse.mybir as mybir
    import concourse.tile as tile

    f32 = mybir.dt.float32
    bf16 = mybir.dt.bfloat16
    fp8 = mybir.dt.float8e4
    AF = mybir.ActivationFunctionType

    nc = bass.Bass("TRN2", num_devices=8)

    nodes = nc.dram_tensor("nodes", [D, N], f32, kind="ExternalInput")
    wq = nc.dram_tensor("wq", [D, DK], f32, kind="ExternalInput")
    wk = nc.dram_tensor("wk", [D, DK], f32, kind="ExternalInput")
    wqb = nc.dram_tensor("wqb", [DK, 1], f32, kind="ExternalInput")
    wkb = nc.dram_tensor("wkb", [DK, 1], f32, kind="ExternalInput")
    gw = nc.dram_tensor("gw", [ITERS, D, D], f32, kind="ExternalInput")
    gb = nc.dram_tensor("gb", [ITERS, 2, 128, 1], f32, kind="ExternalInput")
    agg = nc.dram_tensor("agg", [D, D], f32, kind="ExternalInput")
    m0d = nc.dram_tensor("m0", [128, 1], f32, kind="ExternalInput")
    m1d = nc.dram_tensor("m1", [128, 1], f32, kind="ExternalInput")
    part = nc.dram_tensor("part", [RH, D], f32, kind="ExternalOutput")

    with tile.TileContext(nc) as tc:
        from contextlib import ExitStack

        with ExitStack() as ctx:
            const = ctx.enter_context(tc.tile_pool(name="const", bufs=1))

            ones_col = const.tile([128, 2, 16], fp8, name="ones_col")
            nc.vector.memset(ones_col, 1.0)
            ones_row = const.tile([1, 128], f32, name="ones_row")
            nc.vector.memset(ones_row, 1.0)

            # persistent state
            # x^T in bf16, split by feature chunk (dc) and row half (a=mine, b=partner)
            xT = [
                [
                    const.tile([128, RH], bf16, name=f"xT{dc}{hf}")
                    for hf in range(2)
                ]
                for dc in range(2)
            ]
            eP = [const.tile([128, 2, RH], fp8, name=f"eP{i}") for i in range(NCH // 2)]
            rinvB = const.tile([128, RH], f32, name="rinvB")

            # small weights/biases
            wq_s = const.tile([128, 2, DK], bf16, name="wq_s")
            wk_s = const.tile([128, 2, DK], bf16, name="wk_s")
            gw_s = const.tile([128, ITERS, 2, D], bf16, name="gw_s")
            agg_s = const.tile([128, 2, D], bf16, name="agg_s")
            wqb_s = const.tile([128, 1], f32, name="wqb_s")
            wkb_s = const.tile([128, 1], f32, name="wkb_s")
            gb_s = const.tile([128, ITERS, 2, 1], f32, name="gb_s")
            m0_s = const.tile([128, 1], f32, name="m0_s")
            m1_s = const.tile([128, 1], f32, name="m1_s")

            nc.gpsimd.dma_start(out=wqb_s, in_=wqb[:, :])
            nc.gpsimd.dma_start(out=wkb_s, in_=wkb[:, :])
            nc.gpsimd.dma_start(out=m0_s, in_=m0d[:, :])
            nc.gpsimd.dma_start(out=m1_s, in_=m1d[:, :])
            for i in range(ITERS):
                for dc in range(2):
                    nc.gpsimd.dma_start(out=gb_s[:, i, dc, :], in_=gb[i, dc, :, :])

            # pools used from P0 onward
            kq = ctx.enter_context(tc.tile_pool(name="kq", bufs=1))
            kT = kq.tile([128, N], bf16, name="kT")
            qT = kq.tile([128, RH], bf16, name="qT")
            ps_h = ctx.enter_context(tc.tile_pool(name="ps_h", bufs=1, space="PSUM"))

            def scores_exp(mt, chunks=None, pool=None, tag="pss"):
                # mega rowtile of 1024
                if chunks is None:
                    chunks = range(NCH)
                for ncx in chunks:
                    ps = (pool if pool is not None else ps_sc).tile(
                        [128, 2 * RT], f32, name="pss", tag=tag
                    )
                    for j in range(2):
                        nc.tensor.matmul(
                            ps[:, j * RT : (j + 1) * RT],
                            kT[:, ncx * 128 : (ncx + 1) * 128],
                            qT[:, (2 * mt + j) * RT : (2 * mt + j + 1) * RT],
                            start=True,
                            stop=True,
                        )
                    nc.scalar.activation(
                        out=eP[ncx // 2][
                            :, ncx % 2, 2 * mt * RT : (2 * mt + 2) * RT
                        ],
                        in_=ps,
                        func=AF.Exp,
                        scale=SCALE,
                    )

            def kq_gen(ws, bias_s, dst, hf, col):
                ps = ps_tr.tile([128, RT], f32, name="psk", tag="psk", bufs=3)
                for dc in range(2):
                    nc.tensor.matmul(
                        ps,
                        ws[:, dc, :],
                        xT[dc][hf][:, col : col + RT],
                        start=(dc == 0),
                        stop=(dc == 1),
                    )
                dcol = hf * RH + col
                nc.vector.tensor_scalar_add(
                    out=dst[:, dcol : dcol + RT], in0=ps, scalar1=bias_s
                )

            # ---- P0: stage + cast weights, transpose nodes into x^T ----
            with tc.tile_pool(name="stg", bufs=4) as stg, tc.tile_pool(
                name="ps_tr", bufs=3, space="PSUM"
            ) as ps_tr:
                for dc in range(2):
                    ws = stg.tile([128, DK], f32, name="wstg", tag="wstg")
                    nc.gpsimd.dma_start(out=ws, in_=wq[dc * 128 : (dc + 1) * 128, :])
                    nc.vector.tensor_copy(out=wq_s[:, dc, :], in_=ws)
                    ws2 = stg.tile([128, DK], f32, name="wstg2", tag="wstg")
                    nc.gpsimd.dma_start(out=ws2, in_=wk[dc * 128 : (dc + 1) * 128, :])
                    nc.vector.tensor_copy(out=wk_s[:, dc, :], in_=ws2)
                for i in range(ITERS):
                    for dc in range(2):
                        ws = stg.tile([128, D], f32, name="gstg", tag="gstg")
                        nc.gpsimd.dma_start(
                            out=ws, in_=gw[i, dc * 128 : (dc + 1) * 128, :]
                        )
                        nc.vector.tensor_copy(out=gw_s[:, i, dc, :], in_=ws)
                for dc in range(2):
                    ws = stg.tile([128, D], f32, name="astg", tag="gstg")
                    nc.gpsimd.dma_start(out=ws, in_=agg[dc * 128 : (dc + 1) * 128, :])
                    nc.vector.tensor_copy(out=agg_s[:, dc, :], in_=ws)

                # nodes arrives pre-transposed from the host: two 2MB DMAs,
                # then cast to bf16 by quarters, generating k/q and the first
                # mega-rowtile of scores as columns become ready
                for q in range(4):
                    hf, base = (0, (q % 2) * 1024) if q < 2 else (1, (q % 2) * 1024)
                    for dc in range(2):
                        nf = stg.tile(
                            [128, 1024], f32, name="nf", tag="nf", bufs=6
                        )
                        nc.sync.dma_start(
                            out=nf,
                            in_=nodes[
                                dc * 128 : (dc + 1) * 128,
                                q * 1024 : (q + 1) * 1024,
                            ],
                        )
                        nc.vector.tensor_copy(
                            out=xT[dc][hf][:, base : base + 1024], in_=nf
                        )
                    for ct in range(2):
                        kq_gen(wk_s, wkb_s, kT, hf, base + ct * RT)
                        if hf == 0:
                            kq_gen(wq_s, wqb_s, qT, 0, base + ct * RT)
                    if q == 1:
                        scores_exp(0, range(0, 16), pool=ps_tr, tag="psk")
                    elif q == 2:
                        scores_exp(0, range(16, 24), pool=ps_tr, tag="psk")
                    elif q == 3:
                        scores_exp(0, range(24, 32), pool=ps_tr, tag="psk")

            # ---- P1 + GCN, rowtile-pipelined ----
            p1ctx = ExitStack()
            ps_sc = p1ctx.enter_context(tc.tile_pool(name="ps_sc", bufs=2, space="PSUM"))
            ps_u = p1ctx.enter_context(tc.tile_pool(name="ps_u", bufs=2, space="PSUM"))
            hP = [const.tile([128, 2, D], fp8, name=f"hP{i}") for i in range(NCH // 2)]
            hP2 = [
                const.tile([128, 2, D], fp8, name=f"hQ{i}") for i in range(NCH // 2)
            ]
            racc = ctx.enter_context(tc.tile_pool(name="racc", bufs=2))
            upd = ctx.enter_context(tc.tile_pool(name="upd", bufs=4))
            dram = ctx.enter_context(tc.tile_pool(name="dram", bufs=1, space="DRAM"))
            # exchange payload: this core's iter-2 h chunks (fp8), 4
            # pair-tiles of [128, 2, D] per half of its rows
            cc_in = [
                dram.tile([4 * 128, 2 * D], fp8, name=f"cc_in{g}") for g in range(2)
            ]
            cc_out = [
                dram.tile([8 * 128, 2 * D], fp8, name=f"cc_out{g}") for g in range(2)
            ]

            def r_reduce(rt):
                # R = sum_n E via DoubleRow ones-matmuls, then broadcast +
                # reciprocal
                ps_row = ps_sc.tile([1, RT], f32, name="psrow", tag="psrow", bufs=1)
                for cp in range(NCH // 2):
                    nc.tensor.matmul(
                        ps_row,
                        ones_col[:, :, 0:1],
                        eP[cp][:, :, rt * RT : (rt + 1) * RT],
                        start=(cp == 0),
                        stop=(cp == NCH // 2 - 1),
                        perf_mode=mybir.MatmulPerfMode.DoubleRow,
                    )
                rrow = racc.tile([1, RT], f32, name="rrow", tag="rrow")
                nc.vector.tensor_copy(out=rrow, in_=ps_row)
                ps_b = ps_sc.tile([128, RT], f32, name="psb", tag="psrow", bufs=1)
                nc.tensor.matmul(ps_b, ones_row, rrow, start=True, stop=True)
                nc.vector.reciprocal(
                    out=rinvB[:, rt * RT : (rt + 1) * RT], in_=ps_b
                )

            def h_gen(it, half, rng=None):
                if rng is None:
                    rng = range(HCH) if half == 0 else range(HCH, NCH)
                for ncx in rng:
                    hf, col = (
                        (0, ncx * 128) if ncx < HCH else (1, (ncx - HCH) * 128)
                    )
                    ps = ps_h.tile([128, D], f32, name="psh", tag="psh")
                    for dc in range(2):
                        nc.tensor.matmul(
                            ps,
                            xT[dc][hf][:, col : col + 128],
                            gw_s[:, it, dc, :],
                            start=(dc == 0),
                            stop=(dc == 1),
                        )
                    hdst = hP if it == 0 else hP2
                    nc.scalar.activation(
                        out=hdst[ncx // 2][:, ncx % 2, :], in_=ps, func=AF.Copy
                    )

            def agg_mms(pool, it, rt, cps, pu=None):
                if pu is None:
                    pu = [
                        pool.tile([128, RT], f32, name=f"pu{dc}", tag="pu")
                        for dc in range(2)
                    ]
                hx = hP if it == 0 else hP2
                for cp in cps:
                    for dc in range(2):
                        nc.tensor.matmul(
                            pu[dc],
                            hx[cp][:, :, dc * 128 : (dc + 1) * 128],
                            eP[cp][:, :, rt * RT : (rt + 1) * RT],
                            start=(cp == 0),
                            stop=(cp == NCH // 2 - 1),
                            perf_mode=mybir.MatmulPerfMode.DoubleRow,
                        )
                return pu

            def agg_update(it, rt, pool=None, pu=None):
                if pu is None:
                    pu = agg_mms(pool if pool is not None else ps_u, it, rt, range(NCH // 2))
                else:
                    agg_mms(None, it, rt, range(NCH // 4, NCH // 2), pu=pu)
                for dc in range(2):
                    t = upd.tile([128, RT], f32, name="updt", tag="updt")
                    nc.vector.tensor_mul(
                        t, pu[dc], rinvB[:, rt * RT : (rt + 1) * RT]
                    )
                    nc.scalar.activation(
                        out=t,
                        in_=t,
                        func=AF.Relu,
                        bias=gb_s[:, it, dc, :],
                        scale=1.0,
                    )
                    nc.vector.tensor_add(
                        out=xT[dc][0][:, rt * RT : (rt + 1) * RT],
                        in0=xT[dc][0][:, rt * RT : (rt + 1) * RT],
                        in1=t,
                    )
                if it == 0:
                    # h2 for this rowtile's chunks from the fresh local x1,
                    # staged straight into the exchange buffer
                    h_gen(1, 0, range(4 * rt, 4 * rt + 4))
                    for i, cp in enumerate((2 * rt, 2 * rt + 1)):
                        nc.sync.dma_start(
                            out=cc_in[rt // 2][
                                ((rt % 2) * 2 + i) * 128
                                : ((rt % 2) * 2 + i + 1) * 128,
                                :,
                            ],
                            in_=hP2[cp][:, :, :].rearrange("p a b -> p (a b)"),
                        )

            # pipeline: h1 in the idle head, then scores/exp by 1024-wide
            # mega-rowtiles with [R, agg1, update1] per 512-rowtile behind;
            # the x1 exchange fires in two halves so the first AllGather
            # overlaps the second mega-rowtile of attention compute
            def fire_cc(g):
                nc.gpsimd.collective_compute(
                    "AllGather",
                    mybir.AluOpType.bypass,
                    replica_groups=[[0, 1], [2, 3], [4, 5], [6, 7]],
                    ins=[cc_in[g][:, :].opt()],
                    outs=[cc_out[g][:, :].opt()],
                )

            cct = ctx.enter_context(tc.tile_pool(name="cct", bufs=8))

            def combine(g):
                # place partner h2 pair-tiles into hP[8+4g .. 12+4g]
                for i in range(4):
                    t0 = cct.tile([128, 2 * D], fp8, name="t0", tag="cct")
                    t1 = cct.tile([128, 2 * D], fp8, name="t1", tag="cct")
                    nc.sync.dma_start(
                        out=t0, in_=cc_out[g][i * 128 : (i + 1) * 128, :]
                    )
                    nc.sync.dma_start(
                        out=t1,
                        in_=cc_out[g][(4 + i) * 128 : (5 + i) * 128, :],
                    )
                    nc.vector.tensor_scalar_mul(t0, t0, m1_s)
                    nc.vector.tensor_scalar_mul(t1, t1, m0_s)
                    nc.vector.tensor_add(
                        out=hP2[8 + 4 * g + i][:, :, :].rearrange(
                            "p a b -> p (a b)"
                        ),
                        in0=t0,
                        in1=t1,
                    )

            h_gen(0, 0)
            h_gen(0, 1)
            for mt in range(NRT // 2):
                if mt + 1 < NRT // 2:
                    scores_exp(mt + 1)
                r_reduce(2 * mt)
                r_reduce(2 * mt + 1)
                agg_update(0, 2 * mt)
                agg_update(0, 2 * mt + 1)
                fire_cc(mt)
                if mt == 0:
                    combine(0)

            p1ctx.close()
            combine(1)
            ost = ctx.enter_context(tc.tile_pool(name="ost", bufs=3))

            def out_chunk(rc):
                ps = ps_h.tile([128, D], f32, name="pso", tag="psh")
                for dc in range(2):
                    nc.tensor.matmul(
                        ps,
                        xT[dc][0][:, rc * 128 : (rc + 1) * 128],
                        agg_s[:, dc, :],
                        start=(dc == 0),
                        stop=(dc == 1),
                    )
                ot = ost.tile([128, D], f32, name="ot", tag="ot")
                nc.scalar.activation(out=ot, in_=ps, func=AF.Copy)
                nc.sync.dma_start(out=part[rc * 128 : (rc + 1) * 128, :], in_=ot)

            with tc.tile_pool(name="ps_u2", bufs=6, space="PSUM") as ps_u2:
                pus = [agg_mms(ps_u2, 1, rt, range(NCH // 4)) for rt in range(3)]
                for rt in range(NRT):
                    if rt < 3:
                        agg_update(1, rt, pu=pus[rt])
                    else:
                        agg_update(1, rt, pool=ps_u2)
                    for rc in range(4 * rt, 4 * rt + 4):
                        out_chunk(rc)

    _split_excess_waits(nc, mybir)
    return nc


def _get_nc():
    if "nc" not in _CACHE:
        _CACHE["nc"] = _build()
    return _CACHE["nc"]


def _in_maps(inputs):
    ne = np.asarray(inputs["nodes_embed"], dtype=np.float32)
    wq_w = np.asarray(inputs["WQ_w"], dtype=np.float32)
    wq_b = np.asarray(inputs["WQ_b"], dtype=np.float32)
    wk_w = np.asarray(inputs["WK_w"], dtype=np.float32)
    wk_b = np.asarray(inputs["WK_b"], dtype=np.float32)
    gcn_w = np.asarray(inputs["gcn_W"], dtype=np.float32)
    gcn_b = np.asarray(inputs["gcn_b"], dtype=np.float32)
    agg_w = np.asarray(inputs["agg_W"], dtype=np.float32)

    gb = np.ascontiguousarray(gcn_b.reshape(ITERS, 2, 128, 1))
    maps = []
    for c in range(8):
        b, h, rh = c // 4, (c // 2) % 2, c % 2
        if rh == 0:
            nodes = ne[b]
        else:
            nodes = np.concatenate([ne[b, RH:], ne[b, :RH]], axis=0)
        nodes = np.ascontiguousarray(nodes.T)  # [D, N], device wants x^T
        m0 = np.full((128, 1), 1.0 if rh == 0 else 0.0, np.float32)
        m1 = np.full((128, 1), 0.0 if rh == 0 else 1.0, np.float32)
        maps.append(
            {
                "nodes": np.ascontiguousarray(nodes),
                "wq": np.ascontiguousarray(wq_w[:, h * DK : (h + 1) * DK]),
                "wk": np.ascontiguousarray(wk_w[:, h * DK : (h + 1) * DK]),
                "wqb": np.ascontiguousarray(
                    wq_b[h * DK : (h + 1) * DK].reshape(DK, 1)
                ),
                "wkb": np.ascontiguousarray(
                    wk_b[h * DK : (h + 1) * DK].reshape(DK, 1)
                ),
                "gw": gcn_w,
                "gb": gb,
                "agg": np.ascontiguousarray(agg_w[h * D : (h + 1) * D, :]),
                "m0": m0,
                "m1": m1,
            }
        )
    return maps


def kernel(trace=False, tmpdir=None, **inputs):
    from concourse.bass_utils import run_bass_kernel_spmd

    nc = _get_nc()
    maps = _in_maps(inputs)
    kw = {}
    if trace:
        kw = dict(trace=True, tmpdir=tmpdir)
    res = run_bass_kernel_spmd(nc, maps, core_ids=list(range(8)), **kw)

    agg_b = np.asarray(inputs["agg_b"], dtype=np.float32)
    out = np.zeros((B, N, D), np.float32)
    for b in range(B):
        for rh in range(2):
            rows = slice(rh * RH, (rh + 1) * RH)
            out[b, rows, :] = (
                res.results[4 * b + 0 * 2 + rh]["part"]
                + res.results[4 * b + 1 * 2 + rh]["part"]
                + agg_b
            )
    if trace:
        return out, res
    return out



# revision 14
# speedup vs baseline: 1.0938x; 1.0938x over previous
"""Trainium2 Bass kernel for nn_AttentionGCNLayer (B=2, N=4096, D=256, H=2, ITERS=2).

Sharding: 8 cores = (b in 2) x (h in 2) x (row-half in 2). Each core handles one
(batch, head) pair and one half (2048) of the attention rows, with a pairwise
AllGather of the updated node features between the two GCN iterations.

Schedule (v2): chunk-paced pipeline. The Scalar engine's exp stream (64
activations of [128,1024], ~73us) is the second wall after the PE (~114us);
the program interleaves PE work (scores, h-gen, R rowsums, agg accumulation)
at neighbor-chunk granularity so the PE is never idle waiting on exp:

  A:  kq gen + scores(mega-tile 0) exp-paced, h1 + R(rt0) + agg(rt0)
      partial accumulation interleaved per chunk-pair.
  B:  scores(mega-tile 1) feeding exp(1), R(rt1)/agg(rt1) blocks, then
      chunk-paced R(rt2)/agg(rt2) against the exp(1) drain; h2 + AllGather
      of updated features fire per row-pair as soon as x1 is ready.
  D:  iter-2 aggregation (pair order: cc0-half first, local, cc1-half last
      to match AllGather arrival), output projection + DMA interleaved.

Layout: x kept transposed (x^T [D on 2x128 partitions, N free]) in local row
order; scores computed transposed (E^T = exp(q k^T)^T, neighbors on
partitions) feeding the aggregation matmuls directly; E and h in fp8 with
DoubleRow matmuls; weights arrive pre-cast to bf16 from the host in a single
packed blob (no on-device staging casts). Softmax normalizer R = rowsum(E)
via DoubleRow ones-matmuls; 1/R via the fast DVE reciprocal. Scalar engine
runs exp only; copies/relu/bias live on the Vector engine.
"""

import sys

if "/opt/trn_rl_repo" not in sys.path:
    sys.path.insert(0, "/opt/trn_rl_repo")

import numpy as np

B, N, D, H, ITERS = 2, 4096, 256, 2, 2
DK = D // H                      # 128
RH = N // 2                      # 2048 rows per core
NCH = N // 128                   # 32 neighbor chunks
NCP = NCH // 2                   # 16 neighbor chunk-pairs
RT = 512                         # row tile (one PSUM bank of fp32)
NRT = RH // RT                   # 4 row tiles per core
SCALE = 1.0 / float(np.sqrt(np.float32(DK)))

# packed bf16 weight blob column offsets
WQ0, WK0, GW0, AGG0 = 0, 256, 512, 1536
WBCOLS = 2048

_CACHE = {}


def _seq_engines(mybir):
    return {
        mybir.EngineType.PE,
        mybir.EngineType.Activation,
        mybir.EngineType.Pool,
        mybir.EngineType.DVE,
        mybir.EngineType.SP,
    }


def _split_excess_waits(nc, mybir, max_waits=1):
    """This container's walrus accepts at most one sync-wait per engine
    instruction; hoist extra waits onto preceding NoOps on the same engine."""
    seq = _seq_engines(mybir)
    n_new = 0
    for f in nc.m.functions:
        for blk in f.blocks:
            if not any(
                inst.sync_info is not None
                and inst.sync_info.on_wait
                and len(inst.sync_info.on_wait) > max_waits
                and inst.engine in seq
                for inst in blk.instructions
            ):
                continue
            out = []
            for inst in blk.instructions:
                si = inst.sync_info
                if (
                    si is not None
                    and si.on_wait
                    and len(si.on_wait) > max_waits
                    and inst.engine in seq
                ):
                    waits = list(si.on_wait)
                    keep, extra = waits[:max_waits], waits[max_waits:]
                    while extra:
                        chunk, extra = extra[:max_waits], extra[max_waits:]
                        out.append(
                            mybir.InstNoOp(
                                name=f"{inst.name}-ws{n_new}",
                                sync_info=mybir.SyncInfo(on_wait=chunk, on_update=[]),
                                bass_nofuse=True,
                                engine=inst.engine,
                            )
                        )
                        n_new += 1
                    inst.sync_info = mybir.SyncInfo(
                        on_wait=keep, on_update=list(si.on_update)
                    )
                out.append(inst)
            blk.instructions = out
    return n_new


def _build():
    import concourse.bass as bass
    import concourse.mybir as mybir
    import concourse.tile as tile

    f32 = mybir.dt.float32
    bf16 = mybir.dt.bfloat16
    fp8 = mybir.dt.float8e4
    AF = mybir.ActivationFunctionType
    ALU = mybir.AluOpType

    nc = bass.Bass("TRN2", num_devices=8)

    nodes = nc.dram_tensor("nodes", [D, N], bf16, kind="ExternalInput")
    wb = nc.dram_tensor("wb", [128, WBCOLS], bf16, kind="ExternalInput")
    fb = nc.dram_tensor("fb", [128, 8], f32, kind="ExternalInput")
    part = nc.dram_tensor("part", [RH, D], f32, kind="ExternalOutput")

    with tile.TileContext(nc) as tc:
        from contextlib import ExitStack

        with ExitStack() as ctx:
            const = ctx.enter_context(tc.tile_pool(name="const", bufs=1))

            ones_col = const.tile([128, 2, 16], fp8, name="ones_col")
            nc.vector.memset(ones_col, 1.0)
            ones_row = const.tile([1, 128], f32, name="ones_row")
            nc.vector.memset(ones_row, 1.0)

            # persistent state
            xT = [
                [
                    const.tile([128, RH], bf16, name=f"xT{dc}{hf}")
                    for hf in range(2)
                ]
                for dc in range(2)
            ]
            eP = [const.tile([128, 2, RH], fp8, name=f"eP{i}") for i in range(NCP)]
            hP = [const.tile([128, 2, D], fp8, name=f"hP{i}") for i in range(NCP)]
            hP2 = [const.tile([128, 2, D], fp8, name=f"hQ{i}") for i in range(NCP)]
            rinvB = const.tile([128, RH], f32, name="rinvB")

            wb_s = const.tile([128, WBCOLS], bf16, name="wb_s")
            fb_s = const.tile([128, 8], f32, name="fb_s")
            kT = const.tile([128, N], bf16, name="kT")
            qT = const.tile([128, RH], bf16, name="qT")

            def wq_sl(dc):
                return wb_s[:, WQ0 + dc * 128 : WQ0 + (dc + 1) * 128]

            def wk_sl(dc):
                return wb_s[:, WK0 + dc * 128 : WK0 + (dc + 1) * 128]

            def gw_sl(it, dc):
                o = GW0 + (it * 2 + dc) * 256
                return wb_s[:, o : o + 256]

            def agg_sl(dc):
                o = AGG0 + dc * 256
                return wb_s[:, o : o + 256]

            wqb_s = fb_s[:, 0:1]
            wkb_s = fb_s[:, 1:2]

            def gb_sl(it, dc):
                return fb_s[:, 2 + it * 2 + dc : 3 + it * 2 + dc]

            m0_s = fb_s[:, 6:7]
            m1_s = fb_s[:, 7:8]

            # weight + bias loads on the gpsimd DMA queue; nodes on sync
            nc.gpsimd.dma_start(out=fb_s, in_=fb[:, :])
            nc.gpsimd.dma_start(out=wb_s, in_=wb[:, :])

            # phase pools
            ps_u = ctx.enter_context(tc.tile_pool(name="ps_u", bufs=2, space="PSUM"))
            ps_h = ctx.enter_context(tc.tile_pool(name="ps_h", bufs=1, space="PSUM"))
            racc = ctx.enter_context(tc.tile_pool(name="racc", bufs=2))
            upd = ctx.enter_context(tc.tile_pool(name="upd", bufs=4))
            dram = ctx.enter_context(tc.tile_pool(name="dram", bufs=1, space="DRAM"))
            cct = ctx.enter_context(tc.tile_pool(name="cct", bufs=8))

            cc_in = [
                dram.tile([4 * 128, 2 * D], fp8, name=f"cc_in{g}") for g in range(2)
            ]
            cc_out = [
                dram.tile([8 * 128, 2 * D], fp8, name=f"cc_out{g}") for g in range(2)
            ]

            def load_quarter(q):
                hf, base = (q // 2, (q % 2) * 1024)
                for dc in range(2):
                    nc.sync.dma_start(
                        out=xT[dc][hf][:, base : base + 1024],
                        in_=nodes[
                            dc * 128 : (dc + 1) * 128, q * 1024 : (q + 1) * 1024
                        ],
                    )

            def kq_gen(ps_k, wsl, bias_s, dst, hf, col):
                ps = ps_k.tile([128, RT], f32, name="psk", tag="psk")
                for dc in range(2):
                    nc.tensor.matmul(
                        ps,
                        wsl(dc),
                        xT[dc][hf][:, col : col + RT],
                        start=(dc == 0),
                        stop=(dc == 1),
                    )
                dcol = hf * RH + col
                nc.vector.tensor_scalar_add(
                    out=dst[:, dcol : dcol + RT], in0=ps, scalar1=bias_s
                )

            def sc(mt, ncx):
                # one neighbor chunk of transposed scores for mega-rowtile mt,
                # exp'ed into eP on the Scalar engine
                ps = ps_sc.tile([128, 2 * RT], f32, name="pss", tag="pss")
                for j in range(2):
                    nc.tensor.matmul(
                        ps[:, j * RT : (j + 1) * RT],
                        kT[:, ncx * 128 : (ncx + 1) * 128],
                        qT[:, (2 * mt + j) * RT : (2 * mt + j + 1) * RT],
                        start=True,
                        stop=True,
                    )
                nc.scalar.activation(
                    out=eP[ncx // 2][:, ncx % 2, 2 * mt * RT : (2 * mt + 2) * RT],
                    in_=ps,
                    func=AF.Exp,
                    scale=SCALE,
                )

            def h_chunk(it, ncx):
                hf, col = (0, ncx * 128) if ncx < NCP else (1, (ncx - NCP) * 128)
                ps = ps_h.tile([128, D], f32, name="psh", tag="psh")
                for dc in range(2):
                    nc.tensor.matmul(
                        ps,
                        xT[dc][hf][:, col : col + 128],
                        gw_sl(it, dc),
                        start=(dc == 0),
                        stop=(dc == 1),
                    )
                hdst = hP if it == 0 else hP2
                nc.vector.tensor_copy(out=hdst[ncx // 2][:, ncx % 2, :], in_=ps)

            def r_alloc():
                return ps_r.tile([1, RT], f32, name="psrow", tag="psr")

            def r_step(ps_row, rt, cp, start, stop):
                nc.tensor.matmul(
                    ps_row,
                    ones_col[:, :, 0:1],
                    eP[cp][:, :, rt * RT : (rt + 1) * RT],
                    start=start,
                    stop=stop,
                    perf_mode=mybir.MatmulPerfMode.DoubleRow,
                )

            def r_fin(ps_row, rt):
                # broadcast R across partitions on the PE, then 1/x on DVE
                # (the custom-DVE fast reciprocal doesn't codegen in this
                # toolchain)
                rrow = racc.tile([1, RT], f32, name="rrow", tag="rrow")
                nc.vector.tensor_copy(out=rrow, in_=ps_row)
                ps_b = ps_r.tile([128, RT], f32, name="psb", tag="psr")
                nc.tensor.matmul(ps_b, ones_row, rrow, start=True, stop=True)
                nc.vector.reciprocal(
                    out=rinvB[:, rt * RT : (rt + 1) * RT], in_=ps_b
                )

            def agg_alloc():
                return [
                    ps_u.tile([128, RT], f32, name=f"pu{dc}", tag="pu")
                    for dc in range(2)
                ]

            def agg_step(pu, it, rt, cp, start, stop):
                hx = hP if it == 0 else hP2
                for dc in range(2):
                    nc.tensor.matmul(
                        pu[dc],
                        hx[cp][:, :, dc * 128 : (dc + 1) * 128],
                        eP[cp][:, :, rt * RT : (rt + 1) * RT],
                        start=start,
                        stop=stop,
                        perf_mode=mybir.MatmulPerfMode.DoubleRow,
                    )

            def upd_fin(pu, it, rt):
                # x += relu(agg/R + b): mul, fused bias+relu, residual add (DVE)
                for dc in range(2):
                    t = upd.tile([128, RT], f32, name="updt", tag="updt")
                    nc.vector.tensor_mul(
                        t, pu[dc], rinvB[:, rt * RT : (rt + 1) * RT]
                    )
                    nc.vector.tensor_scalar(
                        out=t,
                        in0=t,
                        scalar1=gb_sl(it, dc),
                        scalar2=0.0,
                        op0=ALU.add,
                        op1=ALU.max,
                    )
                    nc.vector.tensor_add(
                        out=xT[dc][0][:, rt * RT : (rt + 1) * RT],
                        in0=xT[dc][0][:, rt * RT : (rt + 1) * RT],
                        in1=t,
                    )

            def h2_dma(rt):
                # stage this rowtile's h2 pair-tiles into the exchange buffer
                for i, cp in enumerate((2 * rt, 2 * rt + 1)):
                    nc.sync.dma_start(
                        out=cc_in[rt // 2][
                            ((rt % 2) * 2 + i) * 128 : ((rt % 2) * 2 + i + 1) * 128,
                            :,
                        ],
                        in_=hP2[cp][:, :, :].rearrange("p a b -> p (a b)"),
                    )

            def fire_cc(g):
                nc.gpsimd.collective_compute(
                    "AllGather",
                    mybir.AluOpType.bypass,
                    replica_groups=[[0, 1], [2, 3], [4, 5], [6, 7]],
                    ins=[cc_in[g][:, :].opt()],
                    outs=[cc_out[g][:, :].opt()],
                )

            def combine(g):
                # place partner h2 pair-tiles into hP2[8+4g .. 12+4g];
                # rank-select via the m0/m1 input masks (2 fused DVE ops)
                for i in range(4):
                    t0 = cct.tile([128, 2 * D], fp8, name="t0", tag="cct")
                    t1 = cct.tile([128, 2 * D], fp8, name="t1", tag="cct")
                    nc.sync.dma_start(
                        out=t0, in_=cc_out[g][i * 128 : (i + 1) * 128, :]
                    )
                    nc.sync.dma_start(
                        out=t1, in_=cc_out[g][(4 + i) * 128 : (5 + i) * 128, :]
                    )
                    nc.vector.tensor_scalar_mul(t0, t0, m1_s)
                    nc.vector.scalar_tensor_tensor(
                        out=hP2[8 + 4 * g + i][:, :, :].rearrange("p a b -> p (a b)"),
                        in0=t1,
                        scalar=m0_s,
                        in1=t0,
                        op0=ALU.mult,
                        op1=ALU.add,
                    )

            # ---------------- phase A ----------------
            # loads + all kq gen (own PSUM pool, closed before scores pools
            # open), then chunk-paced: scores(0) / h1 / R(rt0) / agg0(rt0)
            # interleaved against the exp(0) drain
            load_quarter(0)
            load_quarter(1)
            load_quarter(2)
            load_quarter(3)
            with tc.tile_pool(name="ps_k", bufs=3, space="PSUM") as ps_k:
                for q in range(2):
                    base = q * 1024
                    for ct in range(2):
                        kq_gen(ps_k, wk_sl, wkb_s, kT, 0, base + ct * RT)
                        kq_gen(ps_k, wq_sl, wqb_s, qT, 0, base + ct * RT)
                for q in range(2):
                    base = q * 1024
                    for ct in range(2):
                        kq_gen(ps_k, wk_sl, wkb_s, kT, 1, base + ct * RT)

            p1 = ExitStack()
            ps_sc = p1.enter_context(tc.tile_pool(name="ps_sc", bufs=2, space="PSUM"))
            ps_r = p1.enter_context(tc.tile_pool(name="ps_r", bufs=1, space="PSUM"))

            pr0 = r_alloc()
            pu0 = agg_alloc()

            def ab_tail(c):
                # interleaved consumers trailing the exp stream by 4 chunks
                if c >= 4 and c % 2 == 0:
                    cp = (c - 4) // 2
                    r_step(pr0, 0, cp, start=(cp == 0), stop=False)
                    agg_step(pu0, 0, 0, cp, start=(cp == 0), stop=False)

            for c in range(32):
                sc(0, c)
                h_chunk(0, c)
                ab_tail(c)
            # drain rt0 pair-steps cp=14,15 and finish
            for cp in (14, 15):
                r_step(pr0, 0, cp, start=False, stop=(cp == 15))
                agg_step(pu0, 0, 0, cp, start=False, stop=(cp == 15))
            r_fin(pr0, 0)
            upd_fin(pu0, 0, 0)

            # ---------------- phase B ----------------
            # scores(1) feeds exp(1); rt1 blocks (exp(0) fully ready), then
            # chunk-paced rt2 against exp(1); h2 + cc per row-pair
            for c in range(0, 4):
                sc(1, c)
            pr1 = r_alloc()
            for cp in range(NCP):
                r_step(pr1, 1, cp, start=(cp == 0), stop=(cp == NCP - 1))
            r_fin(pr1, 1)
            for c in range(4, 8):
                sc(1, c)
            pu1 = agg_alloc()
            for cp in range(NCP // 2):
                agg_step(pu1, 0, 1, cp, start=(cp == 0), stop=False)
            for c in range(8, 10):
                sc(1, c)
            for cp in range(NCP // 2, NCP):
                agg_step(pu1, 0, 1, cp, start=False, stop=(cp == NCP - 1))
            upd_fin(pu1, 0, 1)
            # h2 for rowtiles 0/1 interleaved with the scores stream, then
            # the first AllGather fires while exp(1) is still draining
            for c in range(10, 14):
                sc(1, c)
                h_chunk(1, 2 * (c - 10))
                h_chunk(1, 2 * (c - 10) + 1)
                if c == 11:
                    h2_dma(0)
            h2_dma(1)
            fire_cc(0)
            for c in range(14, 16):
                sc(1, c)
            # chunk-paced rt2 against the exp(1) drain (trail the exp
            # stream; paced pairs 0..7, drained pairs 8..15)
            pr2 = r_alloc()
            pu2 = agg_alloc()
            for c in range(16, 32):
                sc(1, c)
                if c % 2 == 1:
                    cp = (c - 17) // 2
                    r_step(pr2, 2, cp, start=(cp == 0), stop=False)
                    agg_step(pu2, 0, 2, cp, start=(cp == 0), stop=False)
            for cp in range(8, NCP):
                r_step(pr2, 2, cp, start=False, stop=(cp == NCP - 1))
                agg_step(pu2, 0, 2, cp, start=False, stop=(cp == NCP - 1))
            r_fin(pr2, 2)
            upd_fin(pu2, 0, 2)
            # R(rt3) block with rowtile-2 h2 chunks interleaved
            pr3 = r_alloc()
            for cp in range(NCP):
                r_step(pr3, 3, cp, start=(cp == 0), stop=(cp == NCP - 1))
                if cp % 4 == 3:
                    h_chunk(1, 8 + cp // 4)
            r_fin(pr3, 3)
            h2_dma(2)
            pu3 = agg_alloc()
            for cp in range(NCP):
                agg_step(pu3, 0, 3, cp, start=(cp == 0), stop=(cp == NCP - 1))
            upd_fin(pu3, 0, 3)
            for ncx in range(12, 16):
                h_chunk(1, ncx)
            h2_dma(3)
            fire_cc(1)
            combine(0)
            combine(1)

            # ---------------- phase D ----------------
            p1.close()
            ps_u2 = ctx.enter_context(tc.tile_pool(name="ps_u2", bufs=2, space="PSUM"))
            pso = ctx.enter_context(tc.tile_pool(name="pso", bufs=3, space="PSUM"))
            ost = ctx.enter_context(tc.tile_pool(name="ost", bufs=4))
            # aggregation pair order matches AllGather arrival: cc0 half,
            # local half, cc1 half
            pair_order = [8, 9, 10, 11] + list(range(8)) + [12, 13, 14, 15]

            def agg1(rt):
                # alternate between two PSUM pools so consecutive rowtile
                # accumulations overlap
                pool = ps_u if rt % 2 == 0 else ps_u2
                pu = [
                    pool.tile([128, RT], f32, name=f"pv{dc}", tag="pu")
                    for dc in range(2)
                ]
                for i, cp in enumerate(pair_order):
                    agg_step(pu, 1, rt, cp, start=(i == 0), stop=(i == NCP - 1))
                return pu

            def out_chunk(rc, qi):
                ps = pso.tile([128, D], f32, name="pso", tag="pso")
                for dc in range(2):
                    nc.tensor.matmul(
                        ps,
                        xT[dc][0][:, rc * 128 : (rc + 1) * 128],
                        agg_sl(dc),
                        start=(dc == 0),
                        stop=(dc == 1),
                    )
                ot = ost.tile([128, D], f32, name="ot", tag="ot")
                nc.vector.tensor_copy(out=ot, in_=ps)
                eng = nc.sync if qi % 2 == 0 else nc.scalar
                eng.dma_start(out=part[rc * 128 : (rc + 1) * 128, :], in_=ot)

            pus = [agg1(0), agg1(1)]
            upd_fin(pus[0], 1, 0)
            for rt in range(NRT):
                if rt + 2 < NRT:
                    pus.append(agg1(rt + 2))
                if rt + 1 < NRT:
                    upd_fin(pus[rt + 1], 1, rt + 1)
                for rc in range(4 * rt, 4 * rt + 4):
                    out_chunk(rc, rc)

    _split_excess_waits(nc, mybir)
    return nc


def _get_nc():
    if "nc" not in _CACHE:
        _CACHE["nc"] = _build()
    return _CACHE["nc"]


def _in_maps(inputs):
    import ml_dtypes

    bf16 = ml_dtypes.bfloat16

    ne = np.asarray(inputs["nodes_embed"], dtype=np.float32)
    wq_w = np.asarray(inputs["WQ_w"], dtype=np.float32)
    wq_b = np.asarray(inputs["WQ_b"], dtype=np.float32)
    wk_w = np.asarray(inputs["WK_w"], dtype=np.float32)
    wk_b = np.asarray(inputs["WK_b"], dtype=np.float32)
    gcn_w = np.asarray(inputs["gcn_W"], dtype=np.float32)
    gcn_b = np.asarray(inputs["gcn_b"], dtype=np.float32)
    agg_w = np.asarray(inputs["agg_W"], dtype=np.float32)

    maps = []
    for c in range(8):
        b, h, rh = c // 4, (c // 2) % 2, c % 2
        if rh == 0:
            nodes = ne[b]
        else:
            nodes = np.concatenate([ne[b, RH:], ne[b, :RH]], axis=0)
        nodes = np.ascontiguousarray(nodes.T).astype(bf16)  # [D, N], x^T

        wq_h = wq_w[:, h * DK : (h + 1) * DK]
        wk_h = wk_w[:, h * DK : (h + 1) * DK]
        agg_h = agg_w[h * D : (h + 1) * D, :]
        wbm = np.zeros((128, WBCOLS), np.float32)
        wbm[:, WQ0 : WQ0 + 128] = wq_h[0:128, :]
        wbm[:, WQ0 + 128 : WQ0 + 256] = wq_h[128:256, :]
        wbm[:, WK0 : WK0 + 128] = wk_h[0:128, :]
        wbm[:, WK0 + 128 : WK0 + 256] = wk_h[128:256, :]
        for it in range(ITERS):
            for dc in range(2):
                o = GW0 + (it * 2 + dc) * 256
                wbm[:, o : o + 256] = gcn_w[it, dc * 128 : (dc + 1) * 128, :]
        for dc in range(2):
            o = AGG0 + dc * 256
            wbm[:, o : o + 256] = agg_h[dc * 128 : (dc + 1) * 128, :]

        fbm = np.zeros((128, 8), np.float32)
        fbm[:, 0] = wq_b[h * DK : (h + 1) * DK]
        fbm[:, 1] = wk_b[h * DK : (h + 1) * DK]
        for it in range(ITERS):
            for dc in range(2):
                fbm[:, 2 + it * 2 + dc] = gcn_b[it, dc * 128 : (dc + 1) * 128]
        fbm[:, 6] = 1.0 if rh == 0 else 0.0
        fbm[:, 7] = 0.0 if rh == 0 else 1.0

        maps.append(
            {
                "nodes": nodes,
                "wb": np.ascontiguousarray(wbm.astype(bf16)),
                "fb": np.ascontiguousarray(fbm),
            }
        )
    return maps


def kernel(trace=False, tmpdir=None, **inputs):
    from concourse.bass_utils import run_bass_kernel_spmd

    nc = _get_nc()
    maps = _in_maps(inputs)
    kw = {}
    if trace:
        kw = dict(trace=True, tmpdir=tmpdir)
    res = run_bass_kernel_spmd(nc, maps, core_ids=list(range(8)), **kw)

    agg_b = np.asarray(inputs["agg_b"], dtype=np.float32)
    out = np.zeros((B, N, D), np.float32)
    for b in range(B):
        for rh in range(2):
            rows = slice(rh * RH, (rh + 1) * RH)
            out[b, rows, :] = (
                res.results[4 * b + 0 * 2 + rh]["part"]
                + res.results[4 * b + 1 * 2 + rh]["part"]
                + agg_b
            )
    if trace:
        return out, res
    return out
